# revision 36
# baseline (speedup 1.0000x reference)
"""Trainium2 Bass kernel for BaseLayerWithLoRA:
    y = x @ W^T + b + (x @ lora_A^T) @ lora_B^T
  x [4,2048,4096] f32, W [4096,4096], b [4096], lora_A [16,4096], lora_B [4096,16]

Sharding: token-parallel across 8 cores (1024 tokens each, full O per core).
No collectives needed; LoRA is computed per-core on its own token slice.

Per-core device program (all matmuls bf16, 128x128 full-array mode):
  Every matmul keeps tile_size (128,128): the lora_A weights are zero-padded
  from 16 to 128 columns and the lora_B^T closer from 16 to 128 rows (with
  the arT moving operand zero-filled on partitions 16-127), so the PE never
  switches tiling mode (a mode switch drains the array, ~100-200ns each).

  wave 0 (startup, kc-outer): while the 32 x^T chunks stream in, each
  arriving chunk feeds 2 phase-A matmuls (arT accumulation) + 6 main
  matmuls (o-tiles 0-2 x both token halves) so the PE saturates instead of
  idling behind the DMA. 8 PSUM banks: 2 phase-A + 6 wave groups.
  steady (ot 3..31): per (ot, h): 32 K-chunk matmuls + 1 lora closer into
  one PSUM bank; eviction adds bias (DVE tensor_scalar_add) and DMAs out.

  DMA: x chunks on the Sync HWDGE queue; weights/bias/at/outputs on the
  Scalar (ACT) HWDGE queue so the two issue streams and rings run in
  parallel during the startup burst.
"""

import sys

if "/opt/trn_rl_repo" not in sys.path:
    sys.path.insert(0, "/opt/trn_rl_repo")

import numpy as np

B, S, I, O, R = 4, 2048, 4096, 4096, 16
NCORES = 8
NTOK = B * S                 # 8192 tokens
TPC = NTOK // NCORES         # 1024 tokens per core
WAVE_OTS = 3                 # o-tiles computed kc-outer during the x stream


def build_nc(tpc=TPC, i_dim=I, o_dim=O, r=R, tok_tile=512, mm_dtype="bfloat16"):
    import concourse.bacc as bacc
    import concourse.mybir as mybir
    import concourse.tile as tile

    KC = i_dim // 128        # contraction chunks
    OT = o_dim // 128        # output-row tiles
    TT = tpc // tok_tile     # token tiles
    WF = KC * 128 + 128      # per-o-tile weight blob free size (W chunk + lora_B^T)
    f32 = mybir.dt.float32
    f32r = getattr(mybir.dt, mm_dtype)

    nc = bacc.Bacc("TRN2", target_bir_lowering=False, debug=False)
    xt = nc.declare_dram_parameter("xt", [KC, 128, tpc], f32r, isOutput=False)
    wt = nc.declare_dram_parameter("wt", [OT, 128, WF], f32r, isOutput=False)
    at = nc.declare_dram_parameter("at", [128, KC, r], f32r, isOutput=False)
    bias = nc.declare_dram_parameter("bias", [128, OT], f32, isOutput=False)
    out = nc.declare_dram_parameter("out", [OT, 128, tpc], f32, isOutput=True)

    def ts(h):
        return slice(h * tok_tile, (h + 1) * tok_tile)

    with tile.TileContext(nc) as tc:
        with (
            tc.tile_pool(name="const", bufs=1) as constp,
            tc.tile_pool(name="xpool", bufs=KC) as xpool,
            tc.tile_pool(name="wpool", bufs=5) as wpool,
            tc.tile_pool(name="opool", bufs=4) as opool,
            tc.tile_pool(name="psum", bufs=8, space="PSUM") as psum_pool,
        ):
            # -- scalar-queue DMAs: lora_A weights, then the three wave W
            # tiles split into an early part (K-chunks 0-7, what the wave's
            # first matmuls need) and the remainder, so each staggered wave
            # group's weights land before its first matmul instead of behind
            # the whole previous W tile.
            at_c = constp.tile([128, KC, r], f32r)
            nc.scalar.dma_start(at_c[:], at[:])
            wave_w = []
            for ot in range(WAVE_OTS):
                w_sb = wpool.tile([128, WF], f32r, tag="wtile", name=f"w{ot}")
                wave_w.append(w_sb)
            wsplit = 8 * 128
            for ot in range(WAVE_OTS):
                nc.scalar.dma_start(
                    wave_w[ot][:, 0:wsplit], wt[ot, :, 0:wsplit]
                )
            for ot in range(WAVE_OTS):
                nc.scalar.dma_start(
                    wave_w[ot][:, wsplit:WF], wt[ot, :, wsplit:WF]
                )
            b_sb = constp.tile([128, OT], f32)
            nc.scalar.dma_start(b_sb[:], bias[:])

            # K-chunk 0 of phase A runs as a full 128x128-mode matmul with
            # zero-padded weight columns: it start=True-initializes the whole
            # phase-A PSUM bank (has_written set on all partitions) so the
            # later col-tiled chunks can accumulate with acc_flags=0 whatever
            # the per-bank clear semantics of start are.
            at0 = constp.tile([128, 128], f32r)
            nc.gpsimd.memset(at0[:], 0)
            nc.vector.tensor_copy(at0[:, 0:r], at_c[:, 0, :])

            # ar4 moving operand for the lora closers: partition rows
            # 32j..32j+15 hold the 4 col-group partial sums of x @ lora_A^T
            # (the closer's 4x-replicated lora_B^T weight rows reduce them),
            # all other rows stay zero.
            ar4_sb = constp.tile([128, tpc], f32r)
            nc.gpsimd.memset(ar4_sb[:], 0)

            # -- sync-queue DMAs: x^T chunks, one tile per 128-row K-chunk
            # (whole chunks: finer splits only serialize the cold DMA pipe's
            # throughput ramp and starve the wave)
            xts = []
            for kc in range(KC):
                x_t = xpool.tile([128, tpc], f32r, tag="xchunk", name=f"xchunk{kc}")
                nc.sync.dma_start(x_t[:], xt[kc])
                xts.append(x_t)

            # -- PE warm-up: the HAM clock gate holds the PE at 1.2 GHz until
            # it has seen ~3.4 us of sustained activity.  The first real
            # matmul is DMA-gated until ~12 us, so burn that idle window on
            # garbage matmuls (uninitialized SBUF -> scratch PSUM, results
            # discarded) to reach 2.4 GHz before real work arrives.
            warm_w = constp.tile([128, 128], f32r)
            nc.gpsimd.memset(warm_w[:], 0)
            warm_x = constp.tile([128, tok_tile], f32r)
            nc.gpsimd.memset(warm_x[:], 0)
            warm_ps = psum_pool.tile([128, tok_tile], f32, tag="ps", name="warm")
            for _ in range(16):
                nc.tensor.matmul(
                    warm_ps[:], warm_w[:], warm_x[:], start=True, stop=True
                )

            # -- wave 0: kc-outer so each arriving chunk feeds the PE.
            # Phase A for chunks 1-31 runs col-tiled (tile_position (0,32j),
            # j = kc%4): 4 concurrent 128x32 matmuls per pack slot, one pack
            # per 4 chunks, each col group j accumulating its kc subset into
            # PSUM partitions 32j..32j+15 of the same bank.
            pas = [
                psum_pool.tile([128, tok_tile], f32, tag="ps", name=f"pa{h}")
                for h in range(TT)
            ]
            # 5 wave groups: o-tiles 0,1 x both halves + o-tile 2 h0.  With 2
            # phase-A banks that is 7 PSUM slots; the warm-up slot frees at
            # ~15us for the ot2-h1 fill group that covers the copy bubble.
            wave_list = [
                (ot, h) for ot in range(WAVE_OTS - 1) for h in range(TT)
            ] + [(WAVE_OTS - 1, 0)]
            wave_ps = {
                (ot, h): psum_pool.tile(
                    [128, tok_tile], f32, tag="ps", name=f"wps{ot}_{h}"
                )
                for (ot, h) in wave_list
            }

            def pha_pack(chunks):
                for h in range(TT):
                    for c in chunks:
                        j = c % 4
                        nc.tensor.matmul(
                            pas[h][32 * j : 32 * j + r, :],
                            at_c[:, c, :],
                            xts[c][:, ts(h)],
                            start=False,
                            stop=(c == 28 + j),
                            tile_position=(0, 32 * j),
                            skip_group_check=True,
                        )

            for kc in range(KC):
                ck = slice(kc * 128, (kc + 1) * 128)
                if kc == 0:
                    for h in range(TT):
                        nc.tensor.matmul(
                            pas[h][:],
                            at0[:],
                            xts[0][:, ts(h)],
                            start=True,
                            stop=False,
                            skip_group_check=True,
                        )
                # ot1/ot2 enter the wave staggered so their first matmul
                # comes after their W tile lands (ot0's W is first in the
                # scalar queue, ot1's ~4 chunk-periods later, ot2's ~8).
                for (ot, h) in wave_list:
                    lag = 4 * ot
                    if kc >= lag:
                        c = kc - lag
                        nc.tensor.matmul(
                            wave_ps[(ot, h)][:],
                            wave_w[ot][:, (c * 128) : (c + 1) * 128],
                            xts[c][:, ts(h)],
                            start=(c == 0),
                            stop=False,
                        )
                if kc >= 8 and kc % 8 == 0:
                    pha_pack(range(kc - 7, kc + 1))
            pha_pack(range(KC - 7, KC))
            # staggered groups catch up on their trailing chunks
            for (ot, h) in wave_list:
                for c in range(KC - 4 * ot, KC):
                    nc.tensor.matmul(
                        wave_ps[(ot, h)][:],
                        wave_w[ot][:, (c * 128) : (c + 1) * 128],
                        xts[c][:, ts(h)],
                        start=False,
                        stop=False,
                    )

            # ot2 h1 runs next so the PE stays busy while DVE moves the
            # phase-A partials out of PSUM.
            fill_ot = WAVE_OTS - 1
            fill_ps = psum_pool.tile([128, tok_tile], f32, tag="ps", name="fps1")
            for kc in range(KC):
                nc.tensor.matmul(
                    fill_ps[:],
                    wave_w[fill_ot][:, kc * 128 : (kc + 1) * 128],
                    xts[kc][:, ts(1)],
                    start=(kc == 0),
                    stop=False,
                )

            # ar4 rows 32j..32j+15 = col-group-j partials (f32 psum -> bf16)
            for h in range(TT):
                for j in range(4):
                    nc.vector.tensor_copy(
                        ar4_sb[32 * j : 32 * j + r, ts(h)],
                        pas[h][32 * j : 32 * j + r, :],
                    )

            # close + evict the wave groups, then the ot2-h1 fill group
            for (ot, h) in wave_list:
                nc.tensor.matmul(
                    wave_ps[(ot, h)][:],
                    wave_w[ot][:, KC * 128 : KC * 128 + 128],
                    ar4_sb[:, ts(h)],
                    start=False,
                    stop=True,
                )
                o_sb = opool.tile([128, tok_tile], f32, tag="otile")
                nc.vector.tensor_scalar_add(
                    o_sb[:], wave_ps[(ot, h)][:], b_sb[:, ot : ot + 1]
                )
                nc.scalar.dma_start(out[ot, :, ts(h)], o_sb[:])
            nc.tensor.matmul(
                fill_ps[:],
                wave_w[fill_ot][:, KC * 128 : KC * 128 + 128],
                ar4_sb[:, ts(1)],
                start=False,
                stop=True,
            )
            o_sb = opool.tile([128, tok_tile], f32, tag="otile")
            nc.vector.tensor_scalar_add(
                o_sb[:], fill_ps[:], b_sb[:, fill_ot : fill_ot + 1]
            )
            nc.scalar.dma_start(out[fill_ot, :, ts(1)], o_sb[:])

            # -- steady state: per o-tile, one W DMA; per (ot, h): 33 matmuls.
            # The last o-tile runs on quarter-token groups so the final
            # eviction + output DMA tail after the last matmul is shorter.
            for ot in range(WAVE_OTS, OT):
                w_sb = wpool.tile([128, WF], f32r, tag="wtile")
                nc.scalar.dma_start(w_sb[:], wt[ot])
                nt = tok_tile if ot < OT - 1 else tok_tile // 2
                for h in range(tpc // nt):
                    tsl = slice(h * nt, (h + 1) * nt)
                    ps = psum_pool.tile([128, tok_tile], f32, tag="ps")
                    for kc in range(KC):
                        nc.tensor.matmul(
                            ps[:, 0:nt],
                            w_sb[:, kc * 128 : (kc + 1) * 128],
                            xts[kc][:, tsl],
                            start=(kc == 0),
                            stop=False,
                        )
                    nc.tensor.matmul(
                        ps[:, 0:nt],
                        w_sb[:, KC * 128 : KC * 128 + 128],
                        ar4_sb[:, tsl],
                        start=False,
                        stop=True,
                    )
                    o_sb = opool.tile([128, tok_tile], f32, tag="otile")
                    nc.vector.tensor_scalar_add(
                        o_sb[:, 0:nt], ps[:, 0:nt], b_sb[:, ot : ot + 1]
                    )
                    nc.scalar.dma_start(out[ot, :, tsl], o_sb[:, 0:nt])
    nc.compile()
    return nc


def prep_inputs(x, W, b, lora_A, lora_B, tpc=TPC, ncores=NCORES,
                mm_dtype="bfloat16"):
    """Host-side layout marshalling (layout + dtype cast only)."""
    import ml_dtypes

    np_mm = np.float32 if mm_dtype == "float32r" else np.dtype(ml_dtypes.bfloat16)
    i_dim, o_dim, r = W.shape[1], W.shape[0], lora_A.shape[0]
    ntok = tpc * ncores
    x = np.ascontiguousarray(x, dtype=np.float32).reshape(ntok, i_dim)
    W = np.ascontiguousarray(W, dtype=np.float32)
    b = np.ascontiguousarray(b, dtype=np.float32)
    lora_A = np.ascontiguousarray(lora_A, dtype=np.float32)
    lora_B = np.ascontiguousarray(lora_B, dtype=np.float32)

    KC, OT = i_dim // 128, o_dim // 128
    WF = KC * 128 + 128
    # wt blob per o-tile: [ki, kc*128+oo] = W[ot*128+oo, kc*128+ki],
    # last 128 cols rows 0:r = lora_B^T slice: [rr, oo] = lora_B[ot*128+oo, rr]
    wtb = np.zeros((OT, 128, WF), dtype=np_mm)
    wtb[:, :, : KC * 128] = (
        W.reshape(OT, 128, KC, 128).transpose(0, 3, 2, 1).reshape(OT, 128, KC * 128)
    ).astype(np_mm)
    lbT = lora_B.reshape(OT, 128, r).transpose(0, 2, 1).astype(np_mm)
    for j in range(4):
        wtb[:, 32 * j : 32 * j + r, KC * 128 :] = lbT
    # at[ki, kc, rr] = lora_A[rr, kc*128+ki] (compact; the device DMAs it
    # into the first r columns of each 128-wide weight-chunk slot)
    at_c = np.ascontiguousarray(
        lora_A.T.reshape(KC, 128, r).transpose(1, 0, 2).astype(np_mm)
    )
    # bias[p, ot] = b[ot*128+p]
    bias = np.ascontiguousarray(b.reshape(OT, 128).T)

    in_maps = []
    for c in range(ncores):
        xc = x[c * tpc : (c + 1) * tpc]  # [tpc, i_dim]
        # xt[kc, ki, t] = xc[t, kc*128+ki]
        xtc = np.ascontiguousarray(
            xc.reshape(tpc, KC, 128).transpose(1, 2, 0).astype(np_mm)
        )
        in_maps.append({"xt": xtc, "wt": wtb, "at": at_c, "bias": bias})
    return in_maps


def assemble_output(results):
    # each core: out[OT, 128, tpc] == y_c^T; tokens are block-sharded
    outT = np.concatenate([r["out"] for r in results], axis=2)  # [OT,128,ntok]
    o_dim = outT.shape[0] * 128
    ntok = outT.shape[2]
    y = outT.reshape(o_dim, ntok).T  # [ntok, o_dim]
    return np.ascontiguousarray(y)


def run(trace=False, trace_kwargs=None, mm_dtype="bfloat16", **inputs):
    from concourse.bass_utils import run_bass_kernel_spmd

    nc = build_nc(mm_dtype=mm_dtype)
    in_maps = prep_inputs(mm_dtype=mm_dtype, **inputs)
    res = run_bass_kernel_spmd(
        nc,
        in_maps,
        list(range(NCORES)),
        trace=trace,
        trace_kwargs=trace_kwargs or {},
    )
    return assemble_output(res.results).reshape(B, S, O), res


def kernel(**inputs):
    y, _ = run(trace=False, **inputs)
    return y


# revision 38
# speedup vs baseline: 1.0081x; 1.0081x over previous
"""Trainium2 Bass kernel for BaseLayerWithLoRA:
    y = x @ W^T + b + (x @ lora_A^T) @ lora_B^T
  x [4,2048,4096] f32, W [4096,4096], b [4096], lora_A [16,4096], lora_B [4096,16]

Sharding: token-parallel across 8 cores (1024 tokens each, full O per core).
No collectives needed; LoRA is computed per-core on its own token slice.

Per-core device program (all matmuls bf16, 128x128 full-array mode):
  Every matmul keeps tile_size (128,128): the lora_A weights are zero-padded
  from 16 to 128 columns and the lora_B^T closer from 16 to 128 rows (with
  the arT moving operand zero-filled on partitions 16-127), so the PE never
  switches tiling mode (a mode switch drains the array, ~100-200ns each).

  wave 0 (startup, kc-outer): while the 32 x^T chunks stream in, each
  arriving chunk feeds 2 phase-A matmuls (arT accumulation) + 6 main
  matmuls (o-tiles 0-2 x both token halves) so the PE saturates instead of
  idling behind the DMA. 8 PSUM banks: 2 phase-A + 6 wave groups.
  steady (ot 3..31): per (ot, h): 32 K-chunk matmuls + 1 lora closer into
  one PSUM bank; eviction adds bias (DVE tensor_scalar_add) and DMAs out.

  DMA: x chunks on the Sync HWDGE queue; weights/bias/at/outputs on the
  Scalar (ACT) HWDGE queue so the two issue streams and rings run in
  parallel during the startup burst.
"""

import sys

if "/opt/trn_rl_repo" not in sys.path:
    sys.path.insert(0, "/opt/trn_rl_repo")

import numpy as np

B, S, I, O, R = 4, 2048, 4096, 4096, 16
NCORES = 8
NTOK = B * S                 # 8192 tokens
TPC = NTOK // NCORES         # 1024 tokens per core
WAVE_OTS = 3                 # o-tiles computed kc-outer during the x stream


def build_nc(tpc=TPC, i_dim=I, o_dim=O, r=R, tok_tile=512, mm_dtype="bfloat16"):
    import concourse.bacc as bacc
    import concourse.mybir as mybir
    import concourse.tile as tile

    KC = i_dim // 128        # contraction chunks
    OT = o_dim // 128        # output-row tiles
    TT = tpc // tok_tile     # token tiles
    WF = KC * 128 + 128      # per-o-tile weight blob free size (W chunk + lora_B^T)
    f32 = mybir.dt.float32
    f32r = getattr(mybir.dt, mm_dtype)

    nc = bacc.Bacc("TRN2", target_bir_lowering=False, debug=False)
    xt = nc.declare_dram_parameter("xt", [KC, 128, tpc], f32r, isOutput=False)
    wt = nc.declare_dram_parameter("wt", [OT, 128, WF], f32r, isOutput=False)
    at = nc.declare_dram_parameter("at", [128, KC, r], f32r, isOutput=False)
    bias = nc.declare_dram_parameter("bias", [128, OT], f32, isOutput=False)
    out = nc.declare_dram_parameter("out", [OT, 128, tpc], f32, isOutput=True)

    def ts(h):
        return slice(h * tok_tile, (h + 1) * tok_tile)

    with tile.TileContext(nc) as tc:
        with (
            tc.tile_pool(name="const", bufs=1) as constp,
            tc.tile_pool(name="xpool", bufs=KC) as xpool,
            tc.tile_pool(name="wpool", bufs=4) as wpool,
            tc.tile_pool(name="opool", bufs=4) as opool,
            tc.tile_pool(name="psum", bufs=8, space="PSUM") as psum_pool,
        ):
            # -- scalar-queue DMAs: lora_A weights, then the three wave W
            # tiles split into an early part (K-chunks 0-7, what the wave's
            # first matmuls need) and the remainder, so each staggered wave
            # group's weights land before its first matmul instead of behind
            # the whole previous W tile.
            at_c = constp.tile([128, KC, r], f32r)
            nc.scalar.dma_start(at_c[:], at[:])
            wave_w = []
            for ot in range(WAVE_OTS):
                w_sb = wpool.tile([128, WF], f32r, tag="wtile", name=f"w{ot}")
                wave_w.append(w_sb)
            wsplit = 8 * 128
            for ot in range(WAVE_OTS):
                nc.scalar.dma_start(
                    wave_w[ot][:, 0:wsplit], wt[ot, :, 0:wsplit]
                )
            for ot in range(WAVE_OTS):
                nc.scalar.dma_start(
                    wave_w[ot][:, wsplit:WF], wt[ot, :, wsplit:WF]
                )
            b_sb = constp.tile([128, OT], f32)
            nc.scalar.dma_start(b_sb[:], bias[:])

            # K-chunk 0 of phase A runs as a full 128x128-mode matmul with
            # zero-padded weight columns: it start=True-initializes the whole
            # phase-A PSUM bank (has_written set on all partitions) so the
            # later col-tiled chunks can accumulate with acc_flags=0 whatever
            # the per-bank clear semantics of start are.
            at0 = constp.tile([128, 128], f32r)
            nc.gpsimd.memset(at0[:], 0)
            nc.vector.tensor_copy(at0[:, 0:r], at_c[:, 0, :])

            # ar4 moving operand for the lora closers: partition rows
            # 32j..32j+15 hold the 4 col-group partial sums of x @ lora_A^T
            # (the closer's 4x-replicated lora_B^T weight rows reduce them),
            # all other rows stay zero.
            ar4_sb = constp.tile([128, tpc], f32r)
            nc.gpsimd.memset(ar4_sb[:], 0)

            # -- sync-queue DMAs: x^T chunks, one tile per 128-row K-chunk
            # (whole chunks: finer splits only serialize the cold DMA pipe's
            # throughput ramp and starve the wave)
            xts = []
            for kc in range(KC):
                x_t = xpool.tile([128, tpc], f32r, tag="xchunk", name=f"xchunk{kc}")
                nc.sync.dma_start(x_t[:], xt[kc])
                xts.append(x_t)

            # -- PE warm-up: the HAM clock gate holds the PE at 1.2 GHz until
            # it has seen ~3.4 us of sustained activity.  The first real
            # matmul is DMA-gated until ~12 us, so burn that idle window on
            # garbage matmuls (uninitialized SBUF -> scratch PSUM, results
            # discarded) to reach 2.4 GHz before real work arrives.
            warm_w = constp.tile([128, 128], f32r)
            nc.gpsimd.memset(warm_w[:], 0)
            warm_x = constp.tile([128, tok_tile], f32r)
            nc.gpsimd.memset(warm_x[:], 0)
            warm_ps = psum_pool.tile([128, tok_tile], f32, tag="ps", name="warm")
            for _ in range(18):
                nc.tensor.matmul(
                    warm_ps[:], warm_w[:], warm_x[:], start=True, stop=True
                )

            # -- wave 0: kc-outer so each arriving chunk feeds the PE.
            # Phase A for chunks 1-31 runs col-tiled (tile_position (0,32j),
            # j = kc%4): 4 concurrent 128x32 matmuls per pack slot, one pack
            # per 4 chunks, each col group j accumulating its kc subset into
            # PSUM partitions 32j..32j+15 of the same bank.
            pas = [
                psum_pool.tile([128, tok_tile], f32, tag="ps", name=f"pa{h}")
                for h in range(TT)
            ]
            # 5 wave groups: o-tiles 0,1 x both halves + o-tile 2 h0.  With 2
            # phase-A banks that is 7 PSUM slots; the warm-up slot frees at
            # ~15us for the ot2-h1 fill group that covers the copy bubble.
            wave_list = [
                (ot, h) for ot in range(WAVE_OTS - 1) for h in range(TT)
            ] + [(WAVE_OTS - 1, 0)]
            wave_ps = {
                (ot, h): psum_pool.tile(
                    [128, tok_tile], f32, tag="ps", name=f"wps{ot}_{h}"
                )
                for (ot, h) in wave_list
            }

            def pha_pack(chunks):
                for h in range(TT):
                    for c in chunks:
                        j = c % 4
                        nc.tensor.matmul(
                            pas[h][32 * j : 32 * j + r, :],
                            at_c[:, c, :],
                            xts[c][:, ts(h)],
                            start=False,
                            stop=(c == 28 + j),
                            tile_position=(0, 32 * j),
                            skip_group_check=True,
                        )

            for kc in range(KC):
                ck = slice(kc * 128, (kc + 1) * 128)
                if kc == 0:
                    for h in range(TT):
                        nc.tensor.matmul(
                            pas[h][:],
                            at0[:],
                            xts[0][:, ts(h)],
                            start=True,
                            stop=False,
                            skip_group_check=True,
                        )
                # ot1/ot2 enter the wave staggered so their first matmul
                # comes after their W tile lands (ot0's W is first in the
                # scalar queue, ot1's ~4 chunk-periods later, ot2's ~8).
                for (ot, h) in wave_list:
                    lag = 4 * ot
                    if kc >= lag:
                        c = kc - lag
                        nc.tensor.matmul(
                            wave_ps[(ot, h)][:],
                            wave_w[ot][:, (c * 128) : (c + 1) * 128],
                            xts[c][:, ts(h)],
                            start=(c == 0),
                            stop=False,
                        )
                if kc >= 4 and kc % 4 == 0:
                    pha_pack(range(kc - 3, kc + 1))
            pha_pack(range(KC - 3, KC))
            # staggered groups catch up on their trailing chunks
            for (ot, h) in wave_list:
                for c in range(KC - 4 * ot, KC):
                    nc.tensor.matmul(
                        wave_ps[(ot, h)][:],
                        wave_w[ot][:, (c * 128) : (c + 1) * 128],
                        xts[c][:, ts(h)],
                        start=False,
                        stop=False,
                    )

            # ot2 h1 runs next so the PE stays busy while DVE moves the
            # phase-A partials out of PSUM.
            fill_ot = WAVE_OTS - 1
            fill_ps = psum_pool.tile([128, tok_tile], f32, tag="ps", name="fps1")
            for kc in range(KC):
                nc.tensor.matmul(
                    fill_ps[:],
                    wave_w[fill_ot][:, kc * 128 : (kc + 1) * 128],
                    xts[kc][:, ts(1)],
                    start=(kc == 0),
                    stop=False,
                )

            # ar4 rows 32j..32j+15 = col-group-j partials (f32 psum -> bf16)
            for h in range(TT):
                for j in range(4):
                    nc.vector.tensor_copy(
                        ar4_sb[32 * j : 32 * j + r, ts(h)],
                        pas[h][32 * j : 32 * j + r, :],
                    )

            # close + evict the wave groups, then the ot2-h1 fill group
            for (ot, h) in wave_list:
                nc.tensor.matmul(
                    wave_ps[(ot, h)][:],
                    wave_w[ot][:, KC * 128 : KC * 128 + 128],
                    ar4_sb[:, ts(h)],
                    start=False,
                    stop=True,
                )
                o_sb = opool.tile([128, tok_tile], f32, tag="otile")
                nc.vector.tensor_scalar_add(
                    o_sb[:], wave_ps[(ot, h)][:], b_sb[:, ot : ot + 1]
                )
                nc.scalar.dma_start(out[ot, :, ts(h)], o_sb[:])
            nc.tensor.matmul(
                fill_ps[:],
                wave_w[fill_ot][:, KC * 128 : KC * 128 + 128],
                ar4_sb[:, ts(1)],
                start=False,
                stop=True,
            )
            o_sb = opool.tile([128, tok_tile], f32, tag="otile")
            nc.vector.tensor_scalar_add(
                o_sb[:], fill_ps[:], b_sb[:, fill_ot : fill_ot + 1]
            )
            nc.scalar.dma_start(out[fill_ot, :, ts(1)], o_sb[:])

            # -- steady state: per o-tile, one W DMA; per (ot, h): 33 matmuls.
            # The last o-tile runs on quarter-token groups so the final
            # eviction + output DMA tail after the last matmul is shorter.
            for ot in range(WAVE_OTS, OT):
                w_sb = wpool.tile([128, WF], f32r, tag="wtile")
                nc.scalar.dma_start(w_sb[:], wt[ot])
                nt = tok_tile if ot < OT - 1 else tok_tile // 2
                for h in range(tpc // nt):
                    tsl = slice(h * nt, (h + 1) * nt)
                    ps = psum_pool.tile([128, tok_tile], f32, tag="ps")
                    for kc in range(KC):
                        nc.tensor.matmul(
                            ps[:, 0:nt],
                            w_sb[:, kc * 128 : (kc + 1) * 128],
                            xts[kc][:, tsl],
                            start=(kc == 0),
                            stop=False,
                        )
                    nc.tensor.matmul(
                        ps[:, 0:nt],
                        w_sb[:, KC * 128 : KC * 128 + 128],
                        ar4_sb[:, tsl],
                        start=False,
                        stop=True,
                    )
                    o_sb = opool.tile([128, tok_tile], f32, tag="otile")
                    nc.vector.tensor_scalar_add(
                        o_sb[:, 0:nt], ps[:, 0:nt], b_sb[:, ot : ot + 1]
                    )
                    nc.scalar.dma_start(out[ot, :, tsl], o_sb[:, 0:nt])
    nc.compile()
    return nc


def prep_inputs(x, W, b, lora_A, lora_B, tpc=TPC, ncores=NCORES,
                mm_dtype="bfloat16"):
    """Host-side layout marshalling (layout + dtype cast only)."""
    import ml_dtypes

    np_mm = np.float32 if mm_dtype == "float32r" else np.dtype(ml_dtypes.bfloat16)
    i_dim, o_dim, r = W.shape[1], W.shape[0], lora_A.shape[0]
    ntok = tpc * ncores
    x = np.ascontiguousarray(x, dtype=np.float32).reshape(ntok, i_dim)
    W = np.ascontiguousarray(W, dtype=np.float32)
    b = np.ascontiguousarray(b, dtype=np.float32)
    lora_A = np.ascontiguousarray(lora_A, dtype=np.float32)
    lora_B = np.ascontiguousarray(lora_B, dtype=np.float32)

    KC, OT = i_dim // 128, o_dim // 128
    WF = KC * 128 + 128
    # wt blob per o-tile: [ki, kc*128+oo] = W[ot*128+oo, kc*128+ki],
    # last 128 cols rows 0:r = lora_B^T slice: [rr, oo] = lora_B[ot*128+oo, rr]
    wtb = np.zeros((OT, 128, WF), dtype=np_mm)
    wtb[:, :, : KC * 128] = (
        W.reshape(OT, 128, KC, 128).transpose(0, 3, 2, 1).reshape(OT, 128, KC * 128)
    ).astype(np_mm)
    lbT = lora_B.reshape(OT, 128, r).transpose(0, 2, 1).astype(np_mm)
    for j in range(4):
        wtb[:, 32 * j : 32 * j + r, KC * 128 :] = lbT
    # at[ki, kc, rr] = lora_A[rr, kc*128+ki] (compact; the device DMAs it
    # into the first r columns of each 128-wide weight-chunk slot)
    at_c = np.ascontiguousarray(
        lora_A.T.reshape(KC, 128, r).transpose(1, 0, 2).astype(np_mm)
    )
    # bias[p, ot] = b[ot*128+p]
    bias = np.ascontiguousarray(b.reshape(OT, 128).T)

    in_maps = []
    for c in range(ncores):
        xc = x[c * tpc : (c + 1) * tpc]  # [tpc, i_dim]
        # xt[kc, ki, t] = xc[t, kc*128+ki]
        xtc = np.ascontiguousarray(
            xc.reshape(tpc, KC, 128).transpose(1, 2, 0).astype(np_mm)
        )
        in_maps.append({"xt": xtc, "wt": wtb, "at": at_c, "bias": bias})
    return in_maps


def assemble_output(results):
    # each core: out[OT, 128, tpc] == y_c^T; tokens are block-sharded
    outT = np.concatenate([r["out"] for r in results], axis=2)  # [OT,128,ntok]
    o_dim = outT.shape[0] * 128
    ntok = outT.shape[2]
    y = outT.reshape(o_dim, ntok).T  # [ntok, o_dim]
    return np.ascontiguousarray(y)


def run(trace=False, trace_kwargs=None, mm_dtype="bfloat16", **inputs):
    from concourse.bass_utils import run_bass_kernel_spmd

    nc = build_nc(mm_dtype=mm_dtype)
    in_maps = prep_inputs(mm_dtype=mm_dtype, **inputs)
    res = run_bass_kernel_spmd(
        nc,
        in_maps,
        list(range(NCORES)),
        trace=trace,
        trace_kwargs=trace_kwargs or {},
    )
    return assemble_output(res.results).reshape(B, S, O), res


def kernel(**inputs):
    y, _ = run(trace=False, **inputs)
    return y
